# revision 1
# baseline (speedup 1.0000x reference)
"""Trainium2 Bass kernel for the KGEncoder RGCN (nn_KGEncoder_14027363188782).

Math (per batch element b, L=5 layers):
    x0 = ent_emb                                             (E, D)
    per layer i:
      y_r   = x @ Wb_x[i,r] + 1 * c[i,r]^T    (E, NB)  where c[i,r] = rel_r @ Wb_rel[i,r]
      Z     = sum_r adj_r @ y_r               (E, NB)  == sup @ Wb[i]  (deg term folded via c)
      h     = relu(Z @ Ww[i] + bias[i])
      g     = sigmoid(h @ Wh[i] + bh[i])
      x     = x + g * (h - x)
    out_b = sum_e x[e] * m[e] / max(sum_e m[e], 1)

Sharding: core c handles b = c // 2 (pair-replicated, no collectives).
adj is shipped pre-transposed (j-major) in bf16 (exact for 0/1 values).
Big matmul: out Z.T (NB x E) = sum_{r,k} y'[kchunk]_r.T @ adjT_r[kchunk];
NRES relations stay resident in SBUF, the rest stream from HBM each layer.
"""

import numpy as np
import ml_dtypes

import concourse.bacc as bacc
import concourse.bass as bass
import concourse.mybir as mybir
import concourse.tile as tile
from concourse import bass_utils
from concourse.bass import MemorySpace

B, R, E, D, HID, L, NB = 4, 10, 1500, 100, 100, 5, 3
EP = 1536           # entity (j) dim padded to 12*128
CH = EP // 128      # 12 k-chunks
FP8 = True          # fp8 adj (exact for 0/1) -> all relations SBUF-resident
DR = True           # DoubleRow fp8 matmul: 256-deep contraction, 2 elem/lane/cyc
C2 = 6              # 256-row contraction chunks (DoubleRow)
E2 = 1504           # i dim padded to 16-aligned for DoubleRow strides
YQ = 32             # y_all per-chunk col stride (16-aligned)
NRES = 10 if FP8 else 4   # relations resident in SBUF
SG = 3              # k-chunks per streamed stage tile
NW = 500            # psum free-dim chunk (3 per row of E)
RNB = R * NB        # 30
f32 = mybir.dt.float32
bf16 = mybir.dt.bfloat16
ADT = mybir.dt.float8e4 if FP8 else mybir.dt.bfloat16
ADT_NP = ml_dtypes.float8_e4m3fn if FP8 else ml_dtypes.bfloat16
AF = mybir.ActivationFunctionType
AX = mybir.AxisListType

_NC_CACHE = {}


def _build_nc():
    nc = bacc.Bacc("TRN2", target_bir_lowering=False, debug=False)

    if DR:
        adjT = nc.dram_tensor(
            "adjT", [R, C2, 128, 2, E2], ADT, kind="ExternalInput"
        ).ap()
    else:
        adjT = nc.dram_tensor("adjT", [R, EP, E], ADT, kind="ExternalInput").ap()
    xT0 = nc.dram_tensor("xT0", [D, E], f32, kind="ExternalInput").ap()
    maskrep = nc.dram_tensor("maskrep", [HID, E], f32, kind="ExternalInput").ap()
    relT = nc.dram_tensor("relT", [D, R], f32, kind="ExternalInput").ap()
    wbxD = nc.dram_tensor("wbx", [L, D, RNB], f32, kind="ExternalInput").ap()
    wbrD = nc.dram_tensor("wbr", [L, D, RNB], f32, kind="ExternalInput").ap()
    wwD = nc.dram_tensor("ww", [L, NB, HID], f32, kind="ExternalInput").ap()
    whD = nc.dram_tensor("wh", [L, HID, HID], f32, kind="ExternalInput").ap()
    biasD = nc.dram_tensor("biasL", [L, HID], f32, kind="ExternalInput").ap()
    bhD = nc.dram_tensor("bhL", [L, HID], f32, kind="ExternalInput").ap()
    graphD = nc.dram_tensor("graph", [HID, 1], f32, kind="ExternalOutput").ap()

    with tile.TileContext(nc) as tc:
        with (
            tc.tile_pool(name="singles", bufs=1) as singles,
            tc.tile_pool(name="resp", bufs=1) as resp,
            tc.tile_pool(name="stagep", bufs=4) as stagep,
            tc.tile_pool(name="ypool", bufs=2) as ypool,
            tc.tile_pool(name="workp", bufs=2) as workp,
            tc.tile_pool(name="psY", bufs=1, space=MemorySpace.PSUM) as psY,
            tc.tile_pool(name="psC", bufs=1, space=MemorySpace.PSUM) as psC,
            tc.tile_pool(name="psB", bufs=1, space=MemorySpace.PSUM) as psB,
        ):
            # ---- persistent state ----
            xT = singles.tile([D, EP], f32, tag="xT", name="xT")
            nc.sync.dma_start(out=xT[:, 0:E], in_=xT0)
            nc.vector.memset(xT[:, E:EP], 0.0)

            ones = singles.tile([1, 128], f32, tag="ones", name="ones")
            nc.vector.memset(ones[:, :], 1.0)

            mask_sb = singles.tile([HID, E], f32, tag="mask", name="mask_sb")
            nc.sync.dma_start(out=mask_sb[:, :], in_=maskrep)

            relT_sb = singles.tile([D, R], f32, tag="relT", name="relT_sb")
            nc.sync.dma_start(out=relT_sb[:, :], in_=relT)

            wbx_sb, wbr_sb, ww_sb, wh_sb, bias_sb, bh_sb = [], [], [], [], [], []
            for i in range(L):
                wx = singles.tile([D, RNB], f32, tag=f"wbx{i}", name=f"wbx{i}")
                nc.sync.dma_start(out=wx[:, :], in_=wbxD[i])
                wbx_sb.append(wx)
                wr = singles.tile([D, RNB], f32, tag=f"wbr{i}", name=f"wbr{i}")
                nc.sync.dma_start(out=wr[:, :], in_=wbrD[i])
                wbr_sb.append(wr)
                wwt = singles.tile([NB, HID], f32, tag=f"ww{i}", name=f"ww{i}")
                nc.sync.dma_start(out=wwt[:, :], in_=wwD[i])
                ww_sb.append(wwt)
                wht = singles.tile([HID, HID], f32, tag=f"wh{i}", name=f"wh{i}")
                nc.sync.dma_start(out=wht[:, :], in_=whD[i])
                wh_sb.append(wht)
                bt = singles.tile([HID, 1], f32, tag=f"bias{i}", name=f"bias{i}")
                nc.sync.dma_start(out=bt[:, :], in_=biasD[i].unsqueeze(1))
                bias_sb.append(bt)
                bht = singles.tile([HID, 1], f32, tag=f"bh{i}", name=f"bh{i}")
                nc.sync.dma_start(out=bht[:, :], in_=bhD[i].unsqueeze(1))
                bh_sb.append(bht)

            # resident adjT relations: tile (128, CH*E), chunk k at cols [k*E, (k+1)*E)
            res_tiles = []
            for r in range(NRES):
                if DR:
                    rt = resp.tile([128, C2 * 2 * E2], ADT,
                                   tag=f"res{r}", name=f"res{r}")
                    nc.sync.dma_start(
                        out=rt[:, :].rearrange("p (c t i) -> p c t i", c=C2, t=2),
                        in_=adjT[r].rearrange("c p t i -> p c t i"),
                    )
                else:
                    rt = resp.tile([128, CH * E], ADT, tag=f"res{r}", name=f"res{r}")
                    nc.sync.dma_start(
                        out=rt[:, :].rearrange("p (k i) -> p k i", k=CH),
                        in_=adjT[r].rearrange("(k p) i -> p k i", p=128),
                    )
                res_tiles.append(rt)

            # ---- layers ----
            for i in range(L):
                # c[i, r, :] = rel_r @ Wb_rel[i, r]   -> psum row 0, cols 3r..3r+3
                psc = psC.tile([1, RNB], f32, tag="c", name=f"psc{i}")
                for r in range(R):
                    nc.tensor.matmul(
                        psc[:, 3 * r : 3 * r + 3],
                        relT_sb[:, r : r + 1],
                        wbr_sb[i][:, 3 * r : 3 * r + 3],
                        start=True, stop=True,
                    )
                c_sb = workp.tile([1, RNB], f32, tag="c_sb", name=f"c_sb{i}", bufs=2)
                nc.scalar.copy(out=c_sb[:, :], in_=psc[:, :])

                # y'[kchunk] = x[kchunk] @ Wbx[i]  + 1 (x) c   -> bf16 (128, RNB) per chunk
                YS = YQ if DR else RNB
                y_all = ypool.tile([128, CH * YS], ADT, tag="y_all", name=f"y_all{i}")
                for k in range(CH):
                    psy = psY.tile([128, RNB], f32, tag="y", name=f"psy{i}_{k}")
                    nc.tensor.matmul(
                        psy[:, :], xT[:, k * 128 : (k + 1) * 128], wbx_sb[i][:, :],
                        start=True, stop=False,
                    )
                    nc.tensor.matmul(
                        psy[:, :], ones[:, :], c_sb[:, :],
                        start=False, stop=True,
                    )
                    nc.scalar.copy(out=y_all[:, k * YS : k * YS + RNB], in_=psy[:, :])

                # Z.T (NB, E) = sum_{r, k} y'_r[k].T @ adjT_r[k]
                # per i-chunk n: accumulate Z chunk, then basis/highway tail on
                # ACT/DVE overlaps the next chunk's PE matmuls
                assert DR
                h_sb = workp.tile([HID, E], f32, tag="h", name=f"h_sb{i}", bufs=1)
                y_view = y_all[:, :].rearrange("p (k q) -> p k q", q=YQ)
                res_views = [
                    res_tiles[r][:, :].rearrange("p (c t i) -> p c t i", c=C2, t=2)
                    for r in range(R)
                ]
                for n in range(3):
                    ns = slice(n * NW, (n + 1) * NW)
                    psz = psB.tile([NB, 512], f32, tag="zz", bufs=2,
                                   name=f"psz{i}_{n}")
                    cnt = 0
                    for r in range(R):
                        for c in range(C2):
                            nc.tensor.matmul(
                                psz[:, 0:NW],
                                y_view[:, 2 * c : 2 * c + 2, 3 * r : 3 * r + 3],
                                res_views[r][:, c, :, ns],
                                start=(cnt == 0),
                                stop=(cnt == R * C2 - 1),
                                perf_mode=mybir.MatmulPerfMode.DoubleRow,
                            )
                            cnt += 1
                    z_sb = workp.tile([NB, NW], f32, tag="z_sb", bufs=2,
                                      name=f"z_sb{i}_{n}")
                    nc.scalar.copy(out=z_sb[:, :], in_=psz[:, 0:NW])
                    psh = psB.tile([HID, 512], f32, tag="hh", bufs=1,
                                   name=f"psh{i}_{n}")
                    nc.tensor.matmul(
                        psh[:, 0:NW], ww_sb[i][:, :], z_sb[:, :],
                        start=True, stop=True,
                    )
                    nc.scalar.activation(
                        h_sb[:, ns], psh[:, 0:NW], AF.Relu, bias=bias_sb[i][:, :],
                    )
                    psg = psB.tile([HID, 512], f32, tag="gg", bufs=1,
                                   name=f"psg{i}_{n}")
                    nc.tensor.matmul(
                        psg[:, 0:NW], wh_sb[i][:, :], h_sb[:, ns],
                        start=True, stop=True,
                    )
                    nc.scalar.activation(
                        psg[:, 0:NW], psg[:, 0:NW], AF.Sigmoid, bias=bh_sb[i][:, :],
                    )
                    # x = x + g * (h - x)  (chunk n)
                    nc.vector.tensor_sub(h_sb[:, ns], h_sb[:, ns], xT[:, ns])
                    nc.vector.tensor_mul(h_sb[:, ns], h_sb[:, ns], psg[:, 0:NW])
                    nc.vector.tensor_add(xT[:, ns], xT[:, ns], h_sb[:, ns])

            # ---- masked mean over entities ----
            xm = workp.tile([HID, E], f32, tag="h", name="xm", bufs=1)
            nc.vector.tensor_mul(xm[:, :], xT[:, 0:E], mask_sb[:, :])
            gsum = workp.tile([HID, 1], f32, tag="gsum", name="gsum", bufs=1)
            nc.vector.reduce_sum(gsum[:, :], xm[:, :], axis=AX.X)
            den = workp.tile([HID, 1], f32, tag="den", name="den", bufs=1)
            nc.vector.reduce_sum(den[:, :], mask_sb[:, :], axis=AX.X)
            nc.vector.tensor_scalar_max(den[:, :], den[:, :], 1.0)
            nc.vector.reciprocal(den[:, :], den[:, :])
            nc.vector.tensor_mul(gsum[:, :], gsum[:, :], den[:, :])
            nc.sync.dma_start(out=graphD, in_=gsum[:, :])

    nc.compile()
    return nc


def get_nc():
    if "nc" not in _NC_CACHE:
        _NC_CACHE["nc"] = _build_nc()
    return _NC_CACHE["nc"]


def make_in_maps(adj, mask_ids, ent_emb, rel_emb, Wb, Ww, bias, Wh, bh):
    adj = np.asarray(adj, dtype=np.float32)
    if DR:
        pad = np.zeros((B, R, EP, E2), dtype=ADT_NP)
        pad[:, :, :E, :E] = adj.transpose(0, 1, 3, 2).astype(ADT_NP)
        # [b, r, c, p, t, i] = adj[b, r, i, j = c*256 + t*128 + p]
        adjT = np.ascontiguousarray(
            pad.reshape(B, R, C2, 2, 128, E2).transpose(0, 1, 2, 4, 3, 5)
        )
    else:
        adjT = np.zeros((B, R, EP, E), dtype=ADT_NP)
        adjT[:, :, :E, :] = adj.transpose(0, 1, 3, 2).astype(ADT_NP)
    entT = np.ascontiguousarray(np.asarray(ent_emb, np.float32).T)
    relTh = np.ascontiguousarray(np.asarray(rel_emb, np.float32).T)
    Wb5 = np.asarray(Wb, np.float32).reshape(L, R, 2, D, NB)
    wbx = np.ascontiguousarray(Wb5[:, :, 0].transpose(0, 2, 1, 3).reshape(L, D, RNB))
    wbr = np.ascontiguousarray(Wb5[:, :, 1].transpose(0, 2, 1, 3).reshape(L, D, RNB))
    maskf = np.asarray(mask_ids).astype(np.float32)
    common = dict(
        xT0=entT, relT=relTh, wbx=wbx, wbr=wbr,
        ww=np.ascontiguousarray(np.asarray(Ww, np.float32)),
        wh=np.ascontiguousarray(np.asarray(Wh, np.float32)),
        biasL=np.ascontiguousarray(np.asarray(bias, np.float32)),
        bhL=np.ascontiguousarray(np.asarray(bh, np.float32)),
    )
    in_maps = []
    for c in range(8):
        b = c // 2
        m = dict(common)
        m["adjT"] = np.ascontiguousarray(adjT[b])
        m["maskrep"] = np.ascontiguousarray(
            np.broadcast_to(maskf[b][None, :], (HID, E))
        )
        in_maps.append(m)
    return in_maps


def run(inputs, trace=False):
    nc = get_nc()
    in_maps = make_in_maps(**{k: np.asarray(v) for k, v in inputs.items()})
    res = bass_utils.run_bass_kernel_spmd(
        nc, in_maps, core_ids=list(range(8)), trace=trace
    )
    out = np.stack(
        [np.asarray(res.results[2 * b]["graph"]).reshape(HID) for b in range(B)]
    ).astype(np.float32)
    return out, res


def kernel(**inputs):
    out, _ = run(inputs, trace=False)
    return out



# revision 36
# speedup vs baseline: 1.5794x; 1.5794x over previous
"""Trainium2 Bass kernel for the KGEncoder RGCN (nn_KGEncoder_14027363188782).

Math (per batch element b, L=5 layers, basis decomposition folded):
    x0 = ent_emb                                            (E, D)
    per layer i:
      y_r  = x @ Wb_x[i,r] + 1 (x) c[i,r]     (E, NB)  c[i,r] = rel_r @ Wb_rel[i,r]
      Z    = sum_r adj_r @ y_r                (E, NB)
      h    = relu(Z @ Ww[i] + bias[i])
      g    = sigmoid(h @ Wh[i] + bh[i])
      x    = x + g * (h - x)
    out_b = sum_e x[e] * m[e] / max(sum_e m[e], 1)

Sharding: core c handles b = c // 2 (pair-replicated, no collectives).

Implementation notes:
  * adj shipped pre-transposed j-major in fp8 (exact for 0/1), tiled
    [IG=3 i-groups of 512][R][C2=6 j-chunks of 256 (DoubleRow)][128][2][512].
  * Z.T (NB x E) accumulated in PSUM per (layer, ig) from fp8 DoubleRow
    matmuls; the i-group-major DMA order lets layer 0 stream the load and
    layer 1 start partial contraction groups before the load finishes.
  * x/h/z/weights in bf16 (y in fp8); per-(l,ig) tail (z->h->g->x) runs on
    ACT/DVE and overlaps the next chunks' PE work. cg-outer emission for
    l>=1 keeps PE fed across layer boundaries.
"""

import numpy as np
import ml_dtypes

import concourse.bacc as bacc
import concourse.mybir as mybir
import concourse.tile as tile
from concourse import bass_utils
from concourse.bass import MemorySpace

B, R, E, D, HID, L, NB = 4, 10, 1500, 100, 100, 5, 3
EP = 1536            # padded entity count (both i and j)
IG = 3               # i-groups
IW = 512             # i-group width
C2 = 6               # 256-deep DoubleRow contraction chunks
CH = 12              # 128-wide k-chunks for y
YQ = 32              # y per-chunk col stride
RNB = R * NB         # 30
f32 = mybir.dt.float32
bf16 = mybir.dt.bfloat16
fp8 = mybir.dt.float8e4
FP8_NP = ml_dtypes.float8_e4m3fn
BF16_NP = ml_dtypes.bfloat16
AF = mybir.ActivationFunctionType
AX = mybir.AxisListType
DR = mybir.MatmulPerfMode.DoubleRow

# packed bf16 weight/state tensor column layout (mask loads separately,
# after the adjacency stream — it is only needed for the final reduce)
C_REL = EP                   # 1536: relT (D x R)
C_WBXC = C_REL + R           # 1546: wbxc per layer (D+1 x YQ)
C_WBR = C_WBXC + YQ * L      # 1706: wbr per layer (D x RNB, YQ stride)
C_WW = C_WBR + YQ * L        # 1866: ww per layer (NB x HID)
C_WH = C_WW + HID * L        # 2366: wh per layer (HID x HID)
WCOLS = C_WH + HID * L + 2   # 2868

_NC_CACHE = {}


def _build_nc():
    nc = bacc.Bacc("TRN2", target_bir_lowering=False, debug=False)

    adjTD = nc.dram_tensor("adjT", [IG, R, C2, 128, 2, IW], fp8,
                           kind="ExternalInput").ap()
    wpackD = nc.dram_tensor("wpack", [D, WCOLS], bf16,
                            kind="ExternalInput").ap()
    maskD = nc.dram_tensor("maskrep", [HID, E], bf16,
                           kind="ExternalInput").ap()
    fpackD = nc.dram_tensor("fpack", [HID, 2 * L], f32,
                            kind="ExternalInput").ap()
    graphD = nc.dram_tensor("graph", [HID, 1], f32, kind="ExternalOutput").ap()

    with tile.TileContext(nc) as tc:
        with (
            tc.tile_pool(name="singles", bufs=1) as singles,
            tc.tile_pool(name="resp", bufs=1) as resp,
            tc.tile_pool(name="ypool", bufs=2) as ypool,
            tc.tile_pool(name="workp", bufs=2) as workp,
            tc.tile_pool(name="psZ", bufs=5, space=MemorySpace.PSUM) as psZ,
            tc.tile_pool(name="psY", bufs=1, space=MemorySpace.PSUM) as psY,
            tc.tile_pool(name="psHG", bufs=2, space=MemorySpace.PSUM) as psHG,
        ):
            # ---- resident adjacency tiles (first DMA goes ahead of the
            # weight pack so the exclusive DMA pipe finishes ~2.5us earlier;
            # weights are only needed once the first block has landed) ----
            res_tiles = [
                resp.tile([128, IG * C2 * 2 * IW], fp8, tag=f"res{r}",
                          name=f"res{r}")
                for r in range(R)
            ]
            res_views = [
                res_tiles[r][:, :].rearrange("p (g c t i) -> p g c t i",
                                             g=IG, c=C2, t=2)
                for r in range(R)
            ]
            nc.sync.dma_start(
                out=res_views[0][:, 0],
                in_=adjTD[0, 0].rearrange("c p t i -> p c t i"),
            )

            # ---- packed small state: 2 DMAs (SP SEQ serializes DMA issue
            # at ~650ns each, so tiny per-tensor DMAs would delay the big
            # adjacency load by ~20us) ----
            wpack = singles.tile([D, WCOLS], bf16, tag="wpack",
                                 name="wpack")
            ones = singles.tile([1, 128], bf16, tag="ones", name="ones")
            nc.vector.memset(ones[:, :], 1.0)
            nc.sync.dma_start(out=wpack[:, :], in_=wpackD)
            fpack = singles.tile([HID, 2 * L], f32, tag="fpack", name="fpack")
            nc.sync.dma_start(out=fpack[:, :], in_=fpackD)

            xTe = wpack[:, 0:EP]
            relT_sb = wpack[0:D, C_REL:C_REL + R]
            wbx_sb = [wpack[:, C_WBXC + YQ * i: C_WBXC + YQ * i + YQ]
                      for i in range(L)]
            c_sbs = [singles.tile([1, YQ], bf16, tag=f"c{i}", name=f"c_sb{i}")
                     for i in range(L)]
            wbr_sb = [wpack[0:D, C_WBR + YQ * i: C_WBR + YQ * i + RNB]
                      for i in range(L)]
            ww_sb = [wpack[0:NB, C_WW + HID * i: C_WW + HID * (i + 1)]
                     for i in range(L)]
            wh_sb = [wpack[0:HID, C_WH + HID * i: C_WH + HID * (i + 1)]
                     for i in range(L)]
            bias_sb = [fpack[:, i: i + 1] for i in range(L)]
            bh_sb = [fpack[:, L + i: L + i + 1] for i in range(L)]

            # ---- rest of the adjacency, i-group-major DMA order so layer 0
            # streams the load; mask goes last (needed only at the end) ----
            for g in range(IG):
                for r in range(R):
                    if g == 0 and r == 0:
                        continue
                    nc.sync.dma_start(
                        out=res_views[r][:, g],
                        in_=adjTD[g, r].rearrange("c p t i -> p c t i"),
                    )
            mask_sb = singles.tile([HID, E], bf16, tag="mask", name="mask_sb")
            nc.sync.dma_start(out=mask_sb[:, :], in_=maskD)

            # ---- per-layer helpers ----
            def emit_c(i):
                """c[i, r, :] = rel_r @ Wb_rel[i, r] -> c_sbs[i] (1, RNB)."""
                psc = psHG.tile([1, YQ], f32, tag="hg", name=f"psc{i}")
                for r in range(R):
                    nc.tensor.matmul(
                        psc[:, 3 * r: 3 * r + 3],
                        relT_sb[:, r: r + 1],
                        wbr_sb[i][:, 3 * r: 3 * r + 3],
                        start=(r == 0), stop=(r == R - 1),
                    )
                nc.vector.tensor_copy(
                    out=c_sbs[i][:, 0:RNB], in_=psc[:, 0:RNB]
                )

            y_tiles = [None, None]  # double-buffered per-layer y (fp8)

            def emit_y(i, g, psy):
                """y chunks for k in [4g, 4g+4): x @ wbx + ones (x) c."""
                for k in range(4 * g, 4 * g + 4):
                    nc.tensor.matmul(
                        psy[:, YQ * k: YQ * k + RNB],
                        xTe[:, 128 * k: 128 * k + 128],
                        wbx_sb[i][:, 0:RNB],
                        start=True, stop=False,
                    )
                    nc.tensor.matmul(
                        psy[:, YQ * k: YQ * k + RNB],
                        ones[:, :], c_sbs[i][:, 0:RNB],
                        start=False, stop=True,
                    )
                nc.vector.tensor_copy(
                    out=y_tiles[i % 2][:, 128 * g: 128 * g + 128],
                    in_=psy[:, 128 * g: 128 * g + 128],
                )

            NHALF = 1            # tail split factor (1 = full width)
            HW2 = IW // NHALF

            def emit_tail(i, g, psz, ynext=None):
                """z -> h -> gate -> x update for i-group g, in 256-wide
                halves so the first half's chain completes early and (via
                ynext=(layer, psy)) unlocks the next layer's matching
                contraction block sooner."""
                z_sb = workp.tile([NB, IW], bf16, tag="z", name=f"z{i}_{g}",
                                  bufs=2)
                for hh in range(NHALF):
                    nc.vector.tensor_copy(
                        out=z_sb[:, HW2 * hh: HW2 * hh + HW2],
                        in_=psz[:, HW2 * hh: HW2 * hh + HW2])
                for hh in range(NHALF):
                    ns = slice(IW * g + HW2 * hh, IW * g + HW2 * hh + HW2)
                    zs = slice(HW2 * hh, HW2 * hh + HW2)
                    psh = psHG.tile([HID, HW2], f32, tag="hg",
                                    name=f"psh{i}_{g}_{hh}")
                    nc.tensor.matmul(psh[:, :], ww_sb[i][:, :], z_sb[:, zs],
                                     start=True, stop=True)
                    h_sb = workp.tile([HID, HW2], bf16, tag="h",
                                      name=f"h{i}_{g}_{hh}", bufs=2)
                    nc.scalar.activation(h_sb[:, :], psh[:, :], AF.Relu,
                                         bias=bias_sb[i][:, :])
                    psg = psHG.tile([HID, HW2], f32, tag="hg",
                                    name=f"psg{i}_{g}_{hh}")
                    nc.tensor.matmul(psg[:, :], wh_sb[i][:, :], h_sb[:, :],
                                     start=True, stop=True)
                    g_sb = workp.tile([HID, HW2], bf16, tag="g",
                                      name=f"g{i}_{g}_{hh}", bufs=2)
                    nc.scalar.activation(g_sb[:, :], psg[:, :], AF.Sigmoid,
                                         bias=bh_sb[i][:, :])
                    # x = x + g * (h - x)
                    nc.vector.tensor_sub(h_sb[:, :], h_sb[:, :],
                                         xTe[0:HID, ns])
                    nc.vector.tensor_mul(h_sb[:, :], h_sb[:, :], g_sb[:, :])
                    nc.vector.tensor_add(xTe[0:HID, ns], xTe[0:HID, ns],
                                         h_sb[:, :])
                    if ynext is not None:
                        ii, psy = ynext
                        nk = 4 // NHALF
                        k2 = 4 * g + nk * hh
                        for k in range(k2, k2 + nk):
                            nc.tensor.matmul(
                                psy[:, YQ * k: YQ * k + RNB],
                                xTe[:, 128 * k: 128 * k + 128],
                                wbx_sb[ii][:, 0:RNB],
                                start=True, stop=False,
                            )
                            nc.tensor.matmul(
                                psy[:, YQ * k: YQ * k + RNB],
                                ones[:, :], c_sbs[ii][:, 0:RNB],
                                start=False, stop=True,
                            )
                        nc.vector.tensor_copy(
                            out=y_tiles[ii % 2][:, YQ * k2: YQ * (k2 + nk)],
                            in_=psy[:, YQ * k2: YQ * (k2 + nk)],
                        )

            def zmm(i, g, c, r, psz, cnt):
                yv = y_tiles[i % 2][:, :].rearrange("p (k q) -> p k q", q=YQ)
                nc.tensor.matmul(
                    psz[:, :],
                    yv[:, 2 * c: 2 * c + 2, 3 * r: 3 * r + 3],
                    res_views[r][:, g, c],
                    start=(cnt == 0), stop=(cnt == C2 * R - 1),
                    perf_mode=DR,
                )

            # ---- layer 0: stream the adjacency load --------------------------
            # y0 (x0 known), then Z per i-group in DMA arrival order (r-outer,
            # c-inner), layer-1 partial groups interleaved as x1 chunks land.
            y_tiles[0] = ypool.tile([128, CH * YQ], fp8, tag="y", name="y0")
            y_tiles[1] = ypool.tile([128, CH * YQ], fp8, tag="y", name="y1")
            for i in range(L):
                emit_c(i)
            psy0 = psY.tile([128, CH * YQ], f32, tag="y", name="psy0")
            for g in range(IG):
                emit_y(0, g, psy0)

            psz0 = [psZ.tile([NB, IW], f32, tag="z", name=f"psz0_{g}")
                    for g in range(IG)]
            psz1 = [None, None, None]
            cnt1 = [0, 0, 0]
            psy1 = None

            def psz1_get(g):
                if psz1[g] is None:
                    psz1[g] = psZ.tile([NB, IW], f32, tag="z",
                                       name=f"psz1_{g}")
                return psz1[g]

            for g in range(IG):
                # layer-0 Z for this i-group, in DMA arrival order
                for r in range(R):
                    for c in range(C2):
                        zmm(0, g, c, r, psz0[g], r * C2 + c)
                    if g == 2:
                        # interleave ready layer-1 work into the phase-2
                        # stream (cg 0/1 for ig2 as its tiles arrive)
                        for c in (0, 1, 2, 3):
                            zmm(1, 2, c, r, psz1_get(2), cnt1[2])
                            cnt1[2] += 1
                # tail -> x1[g] -> y1[g]
                if psy1 is None:
                    psy1 = psY.tile([128, CH * YQ], f32, tag="y", name="psy1")
                emit_tail(0, g, psz0[g], ynext=(1, psy1))
                # layer-1 partial contraction groups now unlocked:
                # after x1[0]: (cg0, ig0); after x1[1]: (cg0 ig1, cg1 ig0/ig1)
                if g == 0:
                    for r in range(R):
                        for c in (0, 1):
                            zmm(1, 0, c, r, psz1_get(0), cnt1[0])
                            cnt1[0] += 1
                elif g == 1:
                    for ig, cs in ((1, (0, 1)), (0, (2, 3)), (1, (2, 3))):
                        for r in range(R):
                            for c in cs:
                                zmm(1, ig, c, r, psz1_get(ig), cnt1[ig])
                                cnt1[ig] += 1

            # finish layer 1: cg2 per i-group, tails interleaved; y(2, g)
            # deferred one block so independent Z matmuls aren't stuck
            # behind the x2 update chain in PE order
            psy_next = psY.tile([128, CH * YQ], f32, tag="y", name="psy2")
            for g in range(IG):
                for r in range(R):
                    for c in (4, 5):
                        zmm(1, g, c, r, psz1_get(g), cnt1[g])
                        cnt1[g] += 1
                emit_tail(1, g, psz1_get(g))
                if g >= 1:
                    emit_y(2, g - 1, psy_next)
            emit_y(2, 2, psy_next)

            # ---- layers 2..4: steady state, cg-outer ------------------------
            parts = []
            for i in range(2, L):
                last = i == L - 1
                pszs = [psZ.tile([NB, IW], f32, tag="z", name=f"psz{i}_{g}")
                        for g in range(IG)]
                cnts = [0, 0, 0]
                for cg in (0, 1):
                    for g in range(IG):
                        for c in (2 * cg, 2 * cg + 1):
                            for r in range(R):
                                zmm(i, g, c, r, pszs[g], cnts[g])
                                cnts[g] += 1
                if not last:
                    psy_next = psY.tile([128, CH * YQ], f32, tag="y",
                                        name=f"psy{i + 1}")
                else:
                    # den = 1 / max(sum(mask), 1) — emitted here so it runs
                    # hidden under layer-4 compute (mask long since arrived)
                    den = workp.tile([HID, 1], f32, tag="den", name="den",
                                     bufs=1)
                    nc.vector.reduce_sum(den[:, :], mask_sb[:, :], axis=AX.X)
                    nc.vector.tensor_scalar_max(den[:, :], den[:, :], 1.0)
                    nc.vector.reciprocal(den[:, :], den[:, :])
                # cg2 per i-group with tails interleaved: the g0 tail chain
                # (z->h->g->x->y) completes while PE streams g1/g2, so the
                # next layer starts with its y ready.
                for g in range(IG):
                    for c in (4, 5):
                        for r in range(R):
                            zmm(i, g, c, r, pszs[g], cnts[g])
                            cnts[g] += 1
                    emit_tail(i, g, pszs[g],
                              ynext=None if last else (i + 1, psy_next))
                    if last:
                        # partial masked sums, hidden under remaining tails
                        w = (E - IW * g) if g == IG - 1 else IW
                        xm = workp.tile([HID, IW], bf16, tag="xm",
                                        name=f"xm{g}", bufs=2)
                        nc.vector.tensor_mul(
                            xm[:, 0:w], xTe[0:HID, IW * g: IW * g + w],
                            mask_sb[:, IW * g: IW * g + w])
                        pt = workp.tile([HID, 1], f32, tag=f"part{g}",
                                        name=f"part{g}", bufs=1)
                        nc.vector.reduce_sum(pt[:, :], xm[:, 0:w], axis=AX.X)
                        parts.append(pt)

            # ---- masked mean over entities ----------------------------------
            nc.vector.tensor_add(parts[0][:, :], parts[0][:, :],
                                 parts[1][:, :])
            nc.vector.tensor_add(parts[0][:, :], parts[0][:, :],
                                 parts[2][:, :])
            nc.vector.tensor_mul(parts[0][:, :], parts[0][:, :], den[:, :])
            nc.sync.dma_start(out=graphD, in_=parts[0][:, :])

    nc.compile()
    return nc


def get_nc():
    if "nc" not in _NC_CACHE:
        _NC_CACHE["nc"] = _build_nc()
    return _NC_CACHE["nc"]


def make_in_maps(adj, mask_ids, ent_emb, rel_emb, Wb, Ww, bias, Wh, bh):
    adj = np.asarray(adj, dtype=np.float32)
    # [b, g, r, c, p, t, w] = adj[b, r, i=512g+w, j=256c+128t+p]
    pad = np.zeros((B, R, EP, EP), dtype=FP8_NP)
    pad[:, :, :E, :E] = adj.astype(FP8_NP)
    adjT = np.ascontiguousarray(
        pad.reshape(B, R, IG, IW, C2, 2, 128).transpose(0, 2, 1, 4, 6, 5, 3)
    )

    wpack = np.zeros((D, WCOLS), dtype=BF16_NP)
    wpack[:D, :E] = np.asarray(ent_emb, np.float32).T.astype(BF16_NP)
    wpack[:D, C_REL:C_REL + R] = (
        np.asarray(rel_emb, np.float32).T.astype(BF16_NP))
    Wb5 = np.asarray(Wb, np.float32).reshape(L, R, 2, D, NB)
    wbx30 = Wb5[:, :, 0].transpose(0, 2, 1, 3).reshape(L, D, RNB)
    wbr30 = Wb5[:, :, 1].transpose(0, 2, 1, 3).reshape(L, D, RNB)
    for i in range(L):
        wpack[:D, C_WBXC + YQ * i: C_WBXC + YQ * i + RNB] = (
            wbx30[i].astype(BF16_NP))
        wpack[:D, C_WBR + YQ * i: C_WBR + YQ * i + RNB] = (
            wbr30[i].astype(BF16_NP))
        wpack[:NB, C_WW + HID * i: C_WW + HID * (i + 1)] = (
            np.asarray(Ww, np.float32)[i].astype(BF16_NP))
        wpack[:HID, C_WH + HID * i: C_WH + HID * (i + 1)] = (
            np.asarray(Wh, np.float32)[i].astype(BF16_NP))

    fpack = np.zeros((HID, 2 * L), dtype=np.float32)
    fpack[:, 0:L] = np.asarray(bias, np.float32).T
    fpack[:, L:2 * L] = np.asarray(bh, np.float32).T

    maskf = np.asarray(mask_ids).astype(np.float32)
    in_maps = []
    for c in range(8):
        b = c // 2
        mrep = np.ascontiguousarray(
            np.broadcast_to(maskf[b][None, :], (HID, E)).astype(BF16_NP))
        in_maps.append(dict(
            adjT=np.ascontiguousarray(adjT[b]), wpack=wpack,
            maskrep=mrep, fpack=fpack))
    return in_maps


def run(inputs, trace=False):
    nc = get_nc()
    in_maps = make_in_maps(**{k: np.asarray(v) for k, v in inputs.items()})
    res = bass_utils.run_bass_kernel_spmd(
        nc, in_maps, core_ids=list(range(8)), trace=trace
    )
    out = np.stack(
        [np.asarray(res.results[2 * b]["graph"]).reshape(HID) for b in range(B)]
    ).astype(np.float32)
    return out, res


def kernel(**inputs):
    out, _ = run(inputs, trace=False)
    return out


# revision 37
# speedup vs baseline: 1.6104x; 1.0196x over previous
"""Trainium2 Bass kernel for the KGEncoder RGCN (nn_KGEncoder_14027363188782).

Math (per batch element b, L=5 layers, basis decomposition folded):
    x0 = ent_emb                                            (E, D)
    per layer i:
      y_r  = x @ Wb_x[i,r] + 1 (x) c[i,r]     (E, NB)  c[i,r] = rel_r @ Wb_rel[i,r]
      Z    = sum_r adj_r @ y_r                (E, NB)
      h    = relu(Z @ Ww[i] + bias[i])
      g    = sigmoid(h @ Wh[i] + bh[i])
      x    = x + g * (h - x)
    out_b = sum_e x[e] * m[e] / max(sum_e m[e], 1)

Sharding: core c handles b = c // 2 (pair-replicated, no collectives).

Implementation notes:
  * adj shipped pre-transposed j-major in fp8 (exact for 0/1), tiled
    [IG=3 i-groups of 512][R][C2=6 j-chunks of 256 (DoubleRow)][128][2][512].
  * Z.T (NB x E) accumulated in PSUM per (layer, ig) from fp8 DoubleRow
    matmuls; the i-group-major DMA order lets layer 0 stream the load and
    layer 1 start partial contraction groups before the load finishes.
  * x/h/z/weights in bf16 (y in fp8); per-(l,ig) tail (z->h->g->x) runs on
    ACT/DVE and overlaps the next chunks' PE work. cg-outer emission for
    l>=1 keeps PE fed across layer boundaries.
"""

import numpy as np
import ml_dtypes

import concourse.bacc as bacc
import concourse.mybir as mybir
import concourse.tile as tile
from concourse import bass_utils
from concourse.bass import MemorySpace

B, R, E, D, HID, L, NB = 4, 10, 1500, 100, 100, 5, 3
EP = 1536            # padded entity count (both i and j)
IG = 3               # i-groups
IW = 512             # i-group width
C2 = 6               # 256-deep DoubleRow contraction chunks
CH = 12              # 128-wide k-chunks for y
YQ = 32              # y per-chunk col stride
RNB = R * NB         # 30
f32 = mybir.dt.float32
bf16 = mybir.dt.bfloat16
fp8 = mybir.dt.float8e4
FP8_NP = ml_dtypes.float8_e4m3fn
BF16_NP = ml_dtypes.bfloat16
AF = mybir.ActivationFunctionType
AX = mybir.AxisListType
DR = mybir.MatmulPerfMode.DoubleRow

# packed bf16 weight/state tensor column layout (mask loads separately,
# after the adjacency stream — it is only needed for the final reduce)
C_REL = EP                   # 1536: relT (D x R)
C_WBXC = C_REL + R           # 1546: wbxc per layer (D+1 x YQ)
C_WBR = C_WBXC + YQ * L      # 1706: wbr per layer (D x RNB, YQ stride)
C_WW = C_WBR + YQ * L        # 1866: ww per layer (NB x HID)
C_WH = C_WW + HID * L        # 2366: wh per layer (HID x HID)
WCOLS = C_WH + HID * L + 2   # 2868

_NC_CACHE = {}


def _build_nc():
    nc = bacc.Bacc("TRN2", target_bir_lowering=False, debug=False)

    adjTD = nc.dram_tensor("adjT", [IG, R, C2, 128, 2, IW], fp8,
                           kind="ExternalInput").ap()
    wpackD = nc.dram_tensor("wpack", [D, WCOLS], bf16,
                            kind="ExternalInput").ap()
    maskD = nc.dram_tensor("maskrep", [HID, E], bf16,
                           kind="ExternalInput").ap()
    fpackD = nc.dram_tensor("fpack", [HID, 2 * L], f32,
                            kind="ExternalInput").ap()
    graphD = nc.dram_tensor("graph", [HID, 1], f32, kind="ExternalOutput").ap()

    with tile.TileContext(nc) as tc:
        with (
            tc.tile_pool(name="singles", bufs=1) as singles,
            tc.tile_pool(name="resp", bufs=1) as resp,
            tc.tile_pool(name="ypool", bufs=2) as ypool,
            tc.tile_pool(name="workp", bufs=2) as workp,
            tc.tile_pool(name="psZ", bufs=5, space=MemorySpace.PSUM) as psZ,
            tc.tile_pool(name="psY", bufs=1, space=MemorySpace.PSUM) as psY,
            tc.tile_pool(name="psHG", bufs=2, space=MemorySpace.PSUM) as psHG,
        ):
            # ---- resident adjacency tiles (first DMA goes ahead of the
            # weight pack so the exclusive DMA pipe finishes ~2.5us earlier;
            # weights are only needed once the first block has landed) ----
            res_tiles = [
                resp.tile([128, IG * C2 * 2 * IW], fp8, tag=f"res{r}",
                          name=f"res{r}")
                for r in range(R)
            ]
            res_views = [
                res_tiles[r][:, :].rearrange("p (g c t i) -> p g c t i",
                                             g=IG, c=C2, t=2)
                for r in range(R)
            ]
            nc.sync.dma_start(
                out=res_views[0][:, 0],
                in_=adjTD[0, 0].rearrange("c p t i -> p c t i"),
            )

            # ---- packed small state: 2 DMAs (SP SEQ serializes DMA issue
            # at ~650ns each, so tiny per-tensor DMAs would delay the big
            # adjacency load by ~20us) ----
            wpack = singles.tile([D, WCOLS], bf16, tag="wpack",
                                 name="wpack")
            ones = singles.tile([1, 128], bf16, tag="ones", name="ones")
            nc.vector.memset(ones[:, :], 1.0)
            nc.sync.dma_start(out=wpack[:, :], in_=wpackD)
            fpack = singles.tile([HID, 2 * L], f32, tag="fpack", name="fpack")
            nc.sync.dma_start(out=fpack[:, :], in_=fpackD)

            xTe = wpack[:, 0:EP]
            relT_sb = wpack[0:D, C_REL:C_REL + R]
            wbx_sb = [wpack[:, C_WBXC + YQ * i: C_WBXC + YQ * i + YQ]
                      for i in range(L)]
            c_sbs = [singles.tile([1, YQ], bf16, tag=f"c{i}", name=f"c_sb{i}")
                     for i in range(L)]
            wbr_sb = [wpack[0:D, C_WBR + YQ * i: C_WBR + YQ * i + RNB]
                      for i in range(L)]
            ww_sb = [wpack[0:NB, C_WW + HID * i: C_WW + HID * (i + 1)]
                     for i in range(L)]
            wh_sb = [wpack[0:HID, C_WH + HID * i: C_WH + HID * (i + 1)]
                     for i in range(L)]
            bias_sb = [fpack[:, i: i + 1] for i in range(L)]
            bh_sb = [fpack[:, L + i: L + i + 1] for i in range(L)]

            # ---- rest of the adjacency, i-group-major DMA order so layer 0
            # streams the load; mask goes last (needed only at the end) ----
            for g in range(IG):
                for r in range(R):
                    if g == 0 and r == 0:
                        continue
                    nc.sync.dma_start(
                        out=res_views[r][:, g],
                        in_=adjTD[g, r].rearrange("c p t i -> p c t i"),
                    )
            mask_sb = singles.tile([HID, E], bf16, tag="mask", name="mask_sb")
            nc.sync.dma_start(out=mask_sb[:, :], in_=maskD)

            # ---- per-layer helpers ----
            def emit_c(i):
                """c[i, r, :] = rel_r @ Wb_rel[i, r] -> c_sbs[i] (1, RNB)."""
                psc = psHG.tile([1, YQ], f32, tag="hg", name=f"psc{i}")
                for r in range(R):
                    nc.tensor.matmul(
                        psc[:, 3 * r: 3 * r + 3],
                        relT_sb[:, r: r + 1],
                        wbr_sb[i][:, 3 * r: 3 * r + 3],
                        start=(r == 0), stop=(r == R - 1),
                    )
                nc.vector.tensor_copy(
                    out=c_sbs[i][:, 0:RNB], in_=psc[:, 0:RNB]
                )

            y_tiles = [None, None]  # double-buffered per-layer y (fp8)

            def emit_y(i, g, psy):
                """y chunks for k in [4g, 4g+4): x @ wbx + ones (x) c."""
                for k in range(4 * g, 4 * g + 4):
                    nc.tensor.matmul(
                        psy[:, YQ * k: YQ * k + RNB],
                        xTe[:, 128 * k: 128 * k + 128],
                        wbx_sb[i][:, 0:RNB],
                        start=True, stop=False,
                    )
                    nc.tensor.matmul(
                        psy[:, YQ * k: YQ * k + RNB],
                        ones[:, :], c_sbs[i][:, 0:RNB],
                        start=False, stop=True,
                    )
                nc.vector.tensor_copy(
                    out=y_tiles[i % 2][:, 128 * g: 128 * g + 128],
                    in_=psy[:, 128 * g: 128 * g + 128],
                )

            NHALF = 1            # tail split factor (1 = full width)
            HW2 = IW // NHALF

            def emit_tail(i, g, psz, ynext=None):
                """z -> h -> gate -> x update for i-group g, in 256-wide
                halves so the first half's chain completes early and (via
                ynext=(layer, psy)) unlocks the next layer's matching
                contraction block sooner."""
                z_sb = workp.tile([NB, IW], bf16, tag="z", name=f"z{i}_{g}",
                                  bufs=2)
                for hh in range(NHALF):
                    nc.vector.tensor_copy(
                        out=z_sb[:, HW2 * hh: HW2 * hh + HW2],
                        in_=psz[:, HW2 * hh: HW2 * hh + HW2])
                for hh in range(NHALF):
                    ns = slice(IW * g + HW2 * hh, IW * g + HW2 * hh + HW2)
                    zs = slice(HW2 * hh, HW2 * hh + HW2)
                    psh = psHG.tile([HID, HW2], f32, tag="hg",
                                    name=f"psh{i}_{g}_{hh}")
                    nc.tensor.matmul(psh[:, :], ww_sb[i][:, :], z_sb[:, zs],
                                     start=True, stop=True)
                    h_sb = workp.tile([HID, HW2], bf16, tag="h",
                                      name=f"h{i}_{g}_{hh}", bufs=2)
                    nc.scalar.activation(h_sb[:, :], psh[:, :], AF.Relu,
                                         bias=bias_sb[i][:, :])
                    psg = psHG.tile([HID, HW2], f32, tag="hg",
                                    name=f"psg{i}_{g}_{hh}")
                    nc.tensor.matmul(psg[:, :], wh_sb[i][:, :], h_sb[:, :],
                                     start=True, stop=True)
                    g_sb = workp.tile([HID, HW2], bf16, tag="g",
                                      name=f"g{i}_{g}_{hh}", bufs=2)
                    nc.scalar.activation(g_sb[:, :], psg[:, :], AF.Sigmoid,
                                         bias=bh_sb[i][:, :])
                    # x = x + g * (h - x)
                    nc.vector.tensor_sub(h_sb[:, :], h_sb[:, :],
                                         xTe[0:HID, ns])
                    nc.vector.tensor_mul(h_sb[:, :], h_sb[:, :], g_sb[:, :])
                    nc.vector.tensor_add(xTe[0:HID, ns], xTe[0:HID, ns],
                                         h_sb[:, :])
                    if ynext is not None:
                        ii, psy = ynext
                        nk = 4 // NHALF
                        k2 = 4 * g + nk * hh
                        for k in range(k2, k2 + nk):
                            nc.tensor.matmul(
                                psy[:, YQ * k: YQ * k + RNB],
                                xTe[:, 128 * k: 128 * k + 128],
                                wbx_sb[ii][:, 0:RNB],
                                start=True, stop=False,
                            )
                            nc.tensor.matmul(
                                psy[:, YQ * k: YQ * k + RNB],
                                ones[:, :], c_sbs[ii][:, 0:RNB],
                                start=False, stop=True,
                            )
                        nc.vector.tensor_copy(
                            out=y_tiles[ii % 2][:, YQ * k2: YQ * (k2 + nk)],
                            in_=psy[:, YQ * k2: YQ * (k2 + nk)],
                        )

            EG2 = E - 2 * IW     # 476 real columns in the last i-group

            def zmm(i, g, c, r, psz, cnt):
                # The last i-group has 36 padded columns; all but the
                # zero-initializing start matmul skip them (the start one
                # writes zeros there via the padded adjacency, keeping the
                # pad region of x exactly 0).
                w = IW if (g < 2 or cnt == 0) else EG2
                yv = y_tiles[i % 2][:, :].rearrange("p (k q) -> p k q", q=YQ)
                nc.tensor.matmul(
                    psz[:, 0:w],
                    yv[:, 2 * c: 2 * c + 2, 3 * r: 3 * r + 3],
                    res_views[r][:, g, c, :, 0:w],
                    start=(cnt == 0), stop=(cnt == C2 * R - 1),
                    perf_mode=DR,
                )

            # ---- layer 0: stream the adjacency load --------------------------
            # y0 (x0 known), then Z per i-group in DMA arrival order (r-outer,
            # c-inner), layer-1 partial groups interleaved as x1 chunks land.
            y_tiles[0] = ypool.tile([128, CH * YQ], fp8, tag="y", name="y0")
            y_tiles[1] = ypool.tile([128, CH * YQ], fp8, tag="y", name="y1")
            for i in range(L):
                emit_c(i)
            psy0 = psY.tile([128, CH * YQ], f32, tag="y", name="psy0")
            for g in range(IG):
                emit_y(0, g, psy0)

            psz0 = [psZ.tile([NB, IW], f32, tag="z", name=f"psz0_{g}")
                    for g in range(IG)]
            psz1 = [None, None, None]
            cnt1 = [0, 0, 0]
            psy1 = None

            def psz1_get(g):
                if psz1[g] is None:
                    psz1[g] = psZ.tile([NB, IW], f32, tag="z",
                                       name=f"psz1_{g}")
                return psz1[g]

            for g in range(IG):
                # layer-0 Z for this i-group, in DMA arrival order
                for r in range(R):
                    for c in range(C2):
                        zmm(0, g, c, r, psz0[g], r * C2 + c)
                    if g == 2:
                        # interleave ready layer-1 work into the phase-2
                        # stream (cg 0/1 for ig2 as its tiles arrive)
                        for c in (0, 1, 2, 3):
                            zmm(1, 2, c, r, psz1_get(2), cnt1[2])
                            cnt1[2] += 1
                # tail -> x1[g] -> y1[g]
                if psy1 is None:
                    psy1 = psY.tile([128, CH * YQ], f32, tag="y", name="psy1")
                emit_tail(0, g, psz0[g], ynext=(1, psy1))
                # layer-1 partial contraction groups now unlocked:
                # after x1[0]: (cg0, ig0); after x1[1]: (cg0 ig1, cg1 ig0/ig1)
                if g == 0:
                    for r in range(R):
                        for c in (0, 1):
                            zmm(1, 0, c, r, psz1_get(0), cnt1[0])
                            cnt1[0] += 1
                elif g == 1:
                    for ig, cs in ((1, (0, 1)), (0, (2, 3)), (1, (2, 3))):
                        for r in range(R):
                            for c in cs:
                                zmm(1, ig, c, r, psz1_get(ig), cnt1[ig])
                                cnt1[ig] += 1

            # finish layer 1: cg2 per i-group, tails interleaved; y(2, g)
            # deferred one block so independent Z matmuls aren't stuck
            # behind the x2 update chain in PE order
            psy_next = psY.tile([128, CH * YQ], f32, tag="y", name="psy2")
            for g in range(IG):
                for r in range(R):
                    for c in (4, 5):
                        zmm(1, g, c, r, psz1_get(g), cnt1[g])
                        cnt1[g] += 1
                emit_tail(1, g, psz1_get(g))
                if g >= 1:
                    emit_y(2, g - 1, psy_next)
            emit_y(2, 2, psy_next)

            # ---- layers 2..4: steady state, cg-outer ------------------------
            parts = []
            for i in range(2, L):
                last = i == L - 1
                pszs = [psZ.tile([NB, IW], f32, tag="z", name=f"psz{i}_{g}")
                        for g in range(IG)]
                cnts = [0, 0, 0]
                for cg in (0, 1):
                    for g in range(IG):
                        for c in (2 * cg, 2 * cg + 1):
                            for r in range(R):
                                zmm(i, g, c, r, pszs[g], cnts[g])
                                cnts[g] += 1
                if not last:
                    psy_next = psY.tile([128, CH * YQ], f32, tag="y",
                                        name=f"psy{i + 1}")
                else:
                    # den = 1 / max(sum(mask), 1) — emitted here so it runs
                    # hidden under layer-4 compute (mask long since arrived)
                    den = workp.tile([HID, 1], f32, tag="den", name="den",
                                     bufs=1)
                    nc.vector.reduce_sum(den[:, :], mask_sb[:, :], axis=AX.X)
                    nc.vector.tensor_scalar_max(den[:, :], den[:, :], 1.0)
                    nc.vector.reciprocal(den[:, :], den[:, :])
                # cg2 per i-group with tails interleaved: the g0 tail chain
                # (z->h->g->x->y) completes while PE streams g1/g2, so the
                # next layer starts with its y ready.
                for g in range(IG):
                    for c in (4, 5):
                        for r in range(R):
                            zmm(i, g, c, r, pszs[g], cnts[g])
                            cnts[g] += 1
                    emit_tail(i, g, pszs[g],
                              ynext=None if last else (i + 1, psy_next))
                    if last:
                        # partial masked sums, hidden under remaining tails
                        w = (E - IW * g) if g == IG - 1 else IW
                        xm = workp.tile([HID, IW], bf16, tag="xm",
                                        name=f"xm{g}", bufs=2)
                        nc.vector.tensor_mul(
                            xm[:, 0:w], xTe[0:HID, IW * g: IW * g + w],
                            mask_sb[:, IW * g: IW * g + w])
                        pt = workp.tile([HID, 1], f32, tag=f"part{g}",
                                        name=f"part{g}", bufs=1)
                        nc.vector.reduce_sum(pt[:, :], xm[:, 0:w], axis=AX.X)
                        parts.append(pt)

            # ---- masked mean over entities ----------------------------------
            nc.vector.tensor_add(parts[0][:, :], parts[0][:, :],
                                 parts[1][:, :])
            nc.vector.tensor_add(parts[0][:, :], parts[0][:, :],
                                 parts[2][:, :])
            nc.vector.tensor_mul(parts[0][:, :], parts[0][:, :], den[:, :])
            nc.sync.dma_start(out=graphD, in_=parts[0][:, :])

    nc.compile()
    return nc


def get_nc():
    if "nc" not in _NC_CACHE:
        _NC_CACHE["nc"] = _build_nc()
    return _NC_CACHE["nc"]


def make_in_maps(adj, mask_ids, ent_emb, rel_emb, Wb, Ww, bias, Wh, bh):
    adj = np.asarray(adj, dtype=np.float32)
    # [b, g, r, c, p, t, w] = adj[b, r, i=512g+w, j=256c+128t+p]
    pad = np.zeros((B, R, EP, EP), dtype=FP8_NP)
    pad[:, :, :E, :E] = adj.astype(FP8_NP)
    adjT = np.ascontiguousarray(
        pad.reshape(B, R, IG, IW, C2, 2, 128).transpose(0, 2, 1, 4, 6, 5, 3)
    )

    wpack = np.zeros((D, WCOLS), dtype=BF16_NP)
    wpack[:D, :E] = np.asarray(ent_emb, np.float32).T.astype(BF16_NP)
    wpack[:D, C_REL:C_REL + R] = (
        np.asarray(rel_emb, np.float32).T.astype(BF16_NP))
    Wb5 = np.asarray(Wb, np.float32).reshape(L, R, 2, D, NB)
    wbx30 = Wb5[:, :, 0].transpose(0, 2, 1, 3).reshape(L, D, RNB)
    wbr30 = Wb5[:, :, 1].transpose(0, 2, 1, 3).reshape(L, D, RNB)
    for i in range(L):
        wpack[:D, C_WBXC + YQ * i: C_WBXC + YQ * i + RNB] = (
            wbx30[i].astype(BF16_NP))
        wpack[:D, C_WBR + YQ * i: C_WBR + YQ * i + RNB] = (
            wbr30[i].astype(BF16_NP))
        wpack[:NB, C_WW + HID * i: C_WW + HID * (i + 1)] = (
            np.asarray(Ww, np.float32)[i].astype(BF16_NP))
        wpack[:HID, C_WH + HID * i: C_WH + HID * (i + 1)] = (
            np.asarray(Wh, np.float32)[i].astype(BF16_NP))

    fpack = np.zeros((HID, 2 * L), dtype=np.float32)
    fpack[:, 0:L] = np.asarray(bias, np.float32).T
    fpack[:, L:2 * L] = np.asarray(bh, np.float32).T

    maskf = np.asarray(mask_ids).astype(np.float32)
    in_maps = []
    for c in range(8):
        b = c // 2
        mrep = np.ascontiguousarray(
            np.broadcast_to(maskf[b][None, :], (HID, E)).astype(BF16_NP))
        in_maps.append(dict(
            adjT=np.ascontiguousarray(adjT[b]), wpack=wpack,
            maskrep=mrep, fpack=fpack))
    return in_maps


def run(inputs, trace=False):
    nc = get_nc()
    in_maps = make_in_maps(**{k: np.asarray(v) for k, v in inputs.items()})
    res = bass_utils.run_bass_kernel_spmd(
        nc, in_maps, core_ids=list(range(8)), trace=trace
    )
    out = np.stack(
        [np.asarray(res.results[2 * b]["graph"]).reshape(HID) for b in range(B)]
    ).astype(np.float32)
    return out, res


def kernel(**inputs):
    out, _ = run(inputs, trace=False)
    return out


# revision 38
# speedup vs baseline: 1.6208x; 1.0065x over previous
"""Trainium2 Bass kernel for the KGEncoder RGCN (nn_KGEncoder_14027363188782).

Math (per batch element b, L=5 layers, basis decomposition folded):
    x0 = ent_emb                                            (E, D)
    per layer i:
      y_r  = x @ Wb_x[i,r] + 1 (x) c[i,r]     (E, NB)  c[i,r] = rel_r @ Wb_rel[i,r]
      Z    = sum_r adj_r @ y_r                (E, NB)
      h    = relu(Z @ Ww[i] + bias[i])
      g    = sigmoid(h @ Wh[i] + bh[i])
      x    = x + g * (h - x)
    out_b = sum_e x[e] * m[e] / max(sum_e m[e], 1)

Sharding: core c handles b = c // 2 (pair-replicated, no collectives).

Implementation notes:
  * adj shipped pre-transposed j-major in fp8 (exact for 0/1), tiled
    [IG=3 i-groups of 512][R][C2=6 j-chunks of 256 (DoubleRow)][128][2][512].
  * Z.T (NB x E) accumulated in PSUM per (layer, ig) from fp8 DoubleRow
    matmuls; the i-group-major DMA order lets layer 0 stream the load and
    layer 1 start partial contraction groups before the load finishes.
  * x/h/z/weights in bf16 (y in fp8); per-(l,ig) tail (z->h->g->x) runs on
    ACT/DVE and overlaps the next chunks' PE work. cg-outer emission for
    l>=1 keeps PE fed across layer boundaries.
"""

import numpy as np
import ml_dtypes

import concourse.bacc as bacc
import concourse.mybir as mybir
import concourse.tile as tile
from concourse import bass_utils
from concourse.bass import MemorySpace

B, R, E, D, HID, L, NB = 4, 10, 1500, 100, 100, 5, 3
EP = 1536            # padded entity count (both i and j)
IG = 3               # i-groups
IW = 512             # i-group width
C2 = 6               # 256-deep DoubleRow contraction chunks
CH = 12              # 128-wide k-chunks for y
YQ = 32              # y per-chunk col stride
RNB = R * NB         # 30
EG2 = E - 2 * IW     # 476 real columns in the last i-group
f32 = mybir.dt.float32
bf16 = mybir.dt.bfloat16
fp8 = mybir.dt.float8e4
FP8_NP = ml_dtypes.float8_e4m3fn
BF16_NP = ml_dtypes.bfloat16
AF = mybir.ActivationFunctionType
AX = mybir.AxisListType
DR = mybir.MatmulPerfMode.DoubleRow

# packed bf16 weight/state tensor column layout (mask loads separately,
# after the adjacency stream — it is only needed for the final reduce)
C_REL = EP                   # 1536: relT (D x R)
C_WBXC = C_REL + R           # 1546: wbxc per layer (D+1 x YQ)
C_WBR = C_WBXC + YQ * L      # 1706: wbr per layer (D x RNB, YQ stride)
C_WW = C_WBR + YQ * L        # 1866: ww per layer (NB x HID)
C_WH = C_WW + HID * L        # 2366: wh per layer (HID x HID)
WCOLS = C_WH + HID * L + 2   # 2868

_NC_CACHE = {}


def _build_nc():
    nc = bacc.Bacc("TRN2", target_bir_lowering=False, debug=False)

    adjAD = nc.dram_tensor("adjA", [2, R, C2, 128, 2, IW], fp8,
                           kind="ExternalInput").ap()
    adjBD = nc.dram_tensor("adjB", [R, C2, 128, 2, EG2], fp8,
                           kind="ExternalInput").ap()
    wpackD = nc.dram_tensor("wpack", [D, WCOLS], bf16,
                            kind="ExternalInput").ap()
    maskD = nc.dram_tensor("maskrep", [HID, E], bf16,
                           kind="ExternalInput").ap()
    fpackD = nc.dram_tensor("fpack", [HID, 2 * L], f32,
                            kind="ExternalInput").ap()
    graphD = nc.dram_tensor("graph", [HID, 1], f32, kind="ExternalOutput").ap()

    with tile.TileContext(nc) as tc:
        with (
            tc.tile_pool(name="singles", bufs=1) as singles,
            tc.tile_pool(name="resp", bufs=1) as resp,
            tc.tile_pool(name="ypool", bufs=2) as ypool,
            tc.tile_pool(name="workp", bufs=2) as workp,
            tc.tile_pool(name="psZ", bufs=5, space=MemorySpace.PSUM) as psZ,
            tc.tile_pool(name="psY", bufs=1, space=MemorySpace.PSUM) as psY,
            tc.tile_pool(name="psHG", bufs=2, space=MemorySpace.PSUM) as psHG,
        ):
            # ---- resident adjacency tiles (first DMA goes ahead of the
            # weight pack so the exclusive DMA pipe finishes ~2.5us earlier;
            # weights are only needed once the first block has landed) ----
            ACOLS = 2 * C2 * 2 * IW
            BCOLS = C2 * 2 * EG2
            res_tiles = [
                resp.tile([128, ACOLS + BCOLS], fp8, tag=f"res{r}",
                          name=f"res{r}")
                for r in range(R)
            ]
            resA = [
                res_tiles[r][:, 0:ACOLS].rearrange(
                    "p (g c t i) -> p g c t i", g=2, c=C2, t=2)
                for r in range(R)
            ]
            resB = [
                res_tiles[r][:, ACOLS:ACOLS + BCOLS].rearrange(
                    "p (c t i) -> p c t i", c=C2, t=2)
                for r in range(R)
            ]

            def adj_dma(g, r):
                if g < 2:
                    nc.sync.dma_start(
                        out=resA[r][:, g],
                        in_=adjAD[g, r].rearrange("c p t i -> p c t i"))
                else:
                    nc.sync.dma_start(
                        out=resB[r],
                        in_=adjBD[r].rearrange("c p t i -> p c t i"))
            adj_dma(0, 0)

            # ---- packed small state: 2 DMAs (SP SEQ serializes DMA issue
            # at ~650ns each, so tiny per-tensor DMAs would delay the big
            # adjacency load by ~20us) ----
            wpack = singles.tile([D, WCOLS], bf16, tag="wpack",
                                 name="wpack")
            ones = singles.tile([1, 128], bf16, tag="ones", name="ones")
            nc.vector.memset(ones[:, :], 1.0)
            nc.sync.dma_start(out=wpack[:, :], in_=wpackD)
            fpack = singles.tile([HID, 2 * L], f32, tag="fpack", name="fpack")
            nc.sync.dma_start(out=fpack[:, :], in_=fpackD)

            xTe = wpack[:, 0:EP]
            relT_sb = wpack[0:D, C_REL:C_REL + R]
            wbx_sb = [wpack[:, C_WBXC + YQ * i: C_WBXC + YQ * i + YQ]
                      for i in range(L)]
            c_sbs = [singles.tile([1, YQ], bf16, tag=f"c{i}", name=f"c_sb{i}")
                     for i in range(L)]
            wbr_sb = [wpack[0:D, C_WBR + YQ * i: C_WBR + YQ * i + RNB]
                      for i in range(L)]
            ww_sb = [wpack[0:NB, C_WW + HID * i: C_WW + HID * (i + 1)]
                     for i in range(L)]
            wh_sb = [wpack[0:HID, C_WH + HID * i: C_WH + HID * (i + 1)]
                     for i in range(L)]
            bias_sb = [fpack[:, i: i + 1] for i in range(L)]
            bh_sb = [fpack[:, L + i: L + i + 1] for i in range(L)]

            # ---- rest of the adjacency, i-group-major DMA order so layer 0
            # streams the load; mask goes last (needed only at the end) ----
            for g in range(IG):
                for r in range(R):
                    if g == 0 and r == 0:
                        continue
                    adj_dma(g, r)
            mask_sb = singles.tile([HID, E], bf16, tag="mask", name="mask_sb")
            nc.sync.dma_start(out=mask_sb[:, :], in_=maskD)

            # ---- per-layer helpers ----
            def emit_c(i):
                """c[i, r, :] = rel_r @ Wb_rel[i, r] -> c_sbs[i] (1, RNB)."""
                psc = psHG.tile([1, YQ], f32, tag="hg", name=f"psc{i}")
                for r in range(R):
                    nc.tensor.matmul(
                        psc[:, 3 * r: 3 * r + 3],
                        relT_sb[:, r: r + 1],
                        wbr_sb[i][:, 3 * r: 3 * r + 3],
                        start=(r == 0), stop=(r == R - 1),
                    )
                nc.vector.tensor_copy(
                    out=c_sbs[i][:, 0:RNB], in_=psc[:, 0:RNB]
                )

            y_tiles = [None, None]  # double-buffered per-layer y (fp8)

            def emit_y(i, g, psy):
                """y chunks for k in [4g, 4g+4): x @ wbx + ones (x) c."""
                for k in range(4 * g, 4 * g + 4):
                    nc.tensor.matmul(
                        psy[:, YQ * k: YQ * k + RNB],
                        xTe[:, 128 * k: 128 * k + 128],
                        wbx_sb[i][:, 0:RNB],
                        start=True, stop=False,
                    )
                    nc.tensor.matmul(
                        psy[:, YQ * k: YQ * k + RNB],
                        ones[:, :], c_sbs[i][:, 0:RNB],
                        start=False, stop=True,
                    )
                nc.vector.tensor_copy(
                    out=y_tiles[i % 2][:, 128 * g: 128 * g + 128],
                    in_=psy[:, 128 * g: 128 * g + 128],
                )

            NHALF = 1            # tail split factor (1 = full width)
            HW2 = IW // NHALF

            def emit_tail(i, g, psz, ynext=None):
                """z -> h -> gate -> x update for i-group g (the last group
                is 476 wide; the x pad region is never written and stays 0
                so the padded y rows meet only zero adjacency columns)."""
                W = IW if g < 2 else EG2
                z_sb = workp.tile([NB, IW], bf16, tag="z", name=f"z{i}_{g}",
                                  bufs=2)
                nc.vector.tensor_copy(out=z_sb[:, 0:W], in_=psz[:, 0:W])
                for hh in range(NHALF):
                    ns = slice(IW * g, IW * g + W)
                    zs = slice(0, W)
                    psh = psHG.tile([HID, HW2], f32, tag="hg",
                                    name=f"psh{i}_{g}_{hh}")
                    nc.tensor.matmul(psh[:, 0:W], ww_sb[i][:, :],
                                     z_sb[:, zs],
                                     start=True, stop=True)
                    h_sb = workp.tile([HID, HW2], bf16, tag="h",
                                      name=f"h{i}_{g}_{hh}", bufs=2)
                    nc.scalar.activation(h_sb[:, 0:W], psh[:, 0:W], AF.Relu,
                                         bias=bias_sb[i][:, :])
                    psg = psHG.tile([HID, HW2], f32, tag="hg",
                                    name=f"psg{i}_{g}_{hh}")
                    nc.tensor.matmul(psg[:, 0:W], wh_sb[i][:, :],
                                     h_sb[:, 0:W],
                                     start=True, stop=True)
                    g_sb = workp.tile([HID, HW2], bf16, tag="g",
                                      name=f"g{i}_{g}_{hh}", bufs=2)
                    nc.scalar.activation(g_sb[:, 0:W], psg[:, 0:W],
                                         AF.Sigmoid,
                                         bias=bh_sb[i][:, :])
                    # x = x + g * (h - x)
                    nc.vector.tensor_sub(h_sb[:, 0:W], h_sb[:, 0:W],
                                         xTe[0:HID, ns])
                    nc.vector.tensor_mul(h_sb[:, 0:W], h_sb[:, 0:W],
                                         g_sb[:, 0:W])
                    nc.vector.tensor_add(xTe[0:HID, ns], xTe[0:HID, ns],
                                         h_sb[:, 0:W])
                    if ynext is not None:
                        ii, psy = ynext
                        nk = 4 // NHALF
                        k2 = 4 * g + nk * hh
                        for k in range(k2, k2 + nk):
                            nc.tensor.matmul(
                                psy[:, YQ * k: YQ * k + RNB],
                                xTe[:, 128 * k: 128 * k + 128],
                                wbx_sb[ii][:, 0:RNB],
                                start=True, stop=False,
                            )
                            nc.tensor.matmul(
                                psy[:, YQ * k: YQ * k + RNB],
                                ones[:, :], c_sbs[ii][:, 0:RNB],
                                start=False, stop=True,
                            )
                        nc.vector.tensor_copy(
                            out=y_tiles[ii % 2][:, YQ * k2: YQ * (k2 + nk)],
                            in_=psy[:, YQ * k2: YQ * (k2 + nk)],
                        )

            def zmm(i, g, c, r, psz, cnt):
                # The last i-group has 36 padded columns; all but the
                # zero-initializing start matmul skip them (the start one
                # writes zeros there via the padded adjacency, keeping the
                # pad region of x exactly 0).
                yv = y_tiles[i % 2][:, :].rearrange("p (k q) -> p k q", q=YQ)
                if g < 2:
                    rhs, w = resA[r][:, g, c], IW
                else:
                    rhs, w = resB[r][:, c], EG2
                nc.tensor.matmul(
                    psz[:, 0:w],
                    yv[:, 2 * c: 2 * c + 2, 3 * r: 3 * r + 3],
                    rhs,
                    start=(cnt == 0), stop=(cnt == C2 * R - 1),
                    perf_mode=DR,
                )

            # ---- layer 0: stream the adjacency load --------------------------
            # y0 (x0 known), then Z per i-group in DMA arrival order (r-outer,
            # c-inner), layer-1 partial groups interleaved as x1 chunks land.
            y_tiles[0] = ypool.tile([128, CH * YQ], fp8, tag="y", name="y0")
            y_tiles[1] = ypool.tile([128, CH * YQ], fp8, tag="y", name="y1")
            for i in range(L):
                emit_c(i)
            psy0 = psY.tile([128, CH * YQ], f32, tag="y", name="psy0")
            for g in range(IG):
                emit_y(0, g, psy0)

            psz0 = [psZ.tile([NB, IW], f32, tag="z", name=f"psz0_{g}")
                    for g in range(IG)]
            psz1 = [None, None, None]
            cnt1 = [0, 0, 0]
            psy1 = None

            def psz1_get(g):
                if psz1[g] is None:
                    psz1[g] = psZ.tile([NB, IW], f32, tag="z",
                                       name=f"psz1_{g}")
                return psz1[g]

            for g in range(IG):
                # layer-0 Z for this i-group, in DMA arrival order
                for r in range(R):
                    for c in range(C2):
                        zmm(0, g, c, r, psz0[g], r * C2 + c)
                    if g == 2:
                        # interleave ready layer-1 work into the phase-2
                        # stream (cg 0/1 for ig2 as its tiles arrive)
                        for c in (0, 1, 2, 3):
                            zmm(1, 2, c, r, psz1_get(2), cnt1[2])
                            cnt1[2] += 1
                # tail -> x1[g] -> y1[g]
                if psy1 is None:
                    psy1 = psY.tile([128, CH * YQ], f32, tag="y", name="psy1")
                emit_tail(0, g, psz0[g], ynext=(1, psy1))
                # layer-1 partial contraction groups now unlocked:
                # after x1[0]: (cg0, ig0); after x1[1]: (cg0 ig1, cg1 ig0/ig1)
                if g == 0:
                    for r in range(R):
                        for c in (0, 1):
                            zmm(1, 0, c, r, psz1_get(0), cnt1[0])
                            cnt1[0] += 1
                elif g == 1:
                    for ig, cs in ((1, (0, 1)), (0, (2, 3)), (1, (2, 3))):
                        for r in range(R):
                            for c in cs:
                                zmm(1, ig, c, r, psz1_get(ig), cnt1[ig])
                                cnt1[ig] += 1

            # finish layer 1: cg2 per i-group, tails interleaved; y(2, g)
            # deferred one block so independent Z matmuls aren't stuck
            # behind the x2 update chain in PE order
            psy_next = psY.tile([128, CH * YQ], f32, tag="y", name="psy2")
            for g in range(IG):
                for r in range(R):
                    for c in (4, 5):
                        zmm(1, g, c, r, psz1_get(g), cnt1[g])
                        cnt1[g] += 1
                emit_tail(1, g, psz1_get(g))
                if g >= 1:
                    emit_y(2, g - 1, psy_next)
            emit_y(2, 2, psy_next)

            # ---- layers 2..4: steady state, cg-outer ------------------------
            parts = []
            for i in range(2, L):
                last = i == L - 1
                pszs = [psZ.tile([NB, IW], f32, tag="z", name=f"psz{i}_{g}")
                        for g in range(IG)]
                cnts = [0, 0, 0]
                for cg in (0, 1):
                    for g in range(IG):
                        for c in (2 * cg, 2 * cg + 1):
                            for r in range(R):
                                zmm(i, g, c, r, pszs[g], cnts[g])
                                cnts[g] += 1
                if not last:
                    psy_next = psY.tile([128, CH * YQ], f32, tag="y",
                                        name=f"psy{i + 1}")
                else:
                    # den = 1 / max(sum(mask), 1) — emitted here so it runs
                    # hidden under layer-4 compute (mask long since arrived)
                    den = workp.tile([HID, 1], f32, tag="den", name="den",
                                     bufs=1)
                    nc.vector.reduce_sum(den[:, :], mask_sb[:, :], axis=AX.X)
                    nc.vector.tensor_scalar_max(den[:, :], den[:, :], 1.0)
                    nc.vector.reciprocal(den[:, :], den[:, :])
                # cg2 per i-group with tails interleaved: the g0 tail chain
                # (z->h->g->x->y) completes while PE streams g1/g2, so the
                # next layer starts with its y ready.
                for g in range(IG):
                    for c in (4, 5):
                        for r in range(R):
                            zmm(i, g, c, r, pszs[g], cnts[g])
                            cnts[g] += 1
                    emit_tail(i, g, pszs[g],
                              ynext=None if last else (i + 1, psy_next))
                    if last:
                        # partial masked sums, hidden under remaining tails
                        w = (E - IW * g) if g == IG - 1 else IW
                        xm = workp.tile([HID, IW], bf16, tag="xm",
                                        name=f"xm{g}", bufs=2)
                        nc.vector.tensor_mul(
                            xm[:, 0:w], xTe[0:HID, IW * g: IW * g + w],
                            mask_sb[:, IW * g: IW * g + w])
                        pt = workp.tile([HID, 1], f32, tag=f"part{g}",
                                        name=f"part{g}", bufs=1)
                        nc.vector.reduce_sum(pt[:, :], xm[:, 0:w], axis=AX.X)
                        parts.append(pt)

            # ---- masked mean over entities ----------------------------------
            nc.vector.tensor_add(parts[0][:, :], parts[0][:, :],
                                 parts[1][:, :])
            nc.vector.tensor_add(parts[0][:, :], parts[0][:, :],
                                 parts[2][:, :])
            nc.vector.tensor_mul(parts[0][:, :], parts[0][:, :], den[:, :])
            nc.sync.dma_start(out=graphD, in_=parts[0][:, :])

    nc.compile()
    return nc


def get_nc():
    if "nc" not in _NC_CACHE:
        _NC_CACHE["nc"] = _build_nc()
    return _NC_CACHE["nc"]


def make_in_maps(adj, mask_ids, ent_emb, rel_emb, Wb, Ww, bias, Wh, bh):
    adj = np.asarray(adj, dtype=np.float32)
    # [b, g, r, c, p, t, w] = adj[b, r, i=512g+w, j=256c+128t+p]
    pad = np.zeros((B, R, EP, EP), dtype=FP8_NP)
    pad[:, :, :E, :E] = adj.astype(FP8_NP)
    full = pad.reshape(B, R, IG, IW, C2, 2, 128)
    adjA = np.ascontiguousarray(
        full[:, :, 0:2].transpose(0, 2, 1, 4, 6, 5, 3))
    # last i-group packed 476 wide (no i padding -> less DMA)
    adjB = np.ascontiguousarray(
        full[:, :, 2, 0:EG2].transpose(0, 1, 3, 5, 4, 2))

    wpack = np.zeros((D, WCOLS), dtype=BF16_NP)
    wpack[:D, :E] = np.asarray(ent_emb, np.float32).T.astype(BF16_NP)
    wpack[:D, C_REL:C_REL + R] = (
        np.asarray(rel_emb, np.float32).T.astype(BF16_NP))
    Wb5 = np.asarray(Wb, np.float32).reshape(L, R, 2, D, NB)
    wbx30 = Wb5[:, :, 0].transpose(0, 2, 1, 3).reshape(L, D, RNB)
    wbr30 = Wb5[:, :, 1].transpose(0, 2, 1, 3).reshape(L, D, RNB)
    for i in range(L):
        wpack[:D, C_WBXC + YQ * i: C_WBXC + YQ * i + RNB] = (
            wbx30[i].astype(BF16_NP))
        wpack[:D, C_WBR + YQ * i: C_WBR + YQ * i + RNB] = (
            wbr30[i].astype(BF16_NP))
        wpack[:NB, C_WW + HID * i: C_WW + HID * (i + 1)] = (
            np.asarray(Ww, np.float32)[i].astype(BF16_NP))
        wpack[:HID, C_WH + HID * i: C_WH + HID * (i + 1)] = (
            np.asarray(Wh, np.float32)[i].astype(BF16_NP))

    fpack = np.zeros((HID, 2 * L), dtype=np.float32)
    fpack[:, 0:L] = np.asarray(bias, np.float32).T
    fpack[:, L:2 * L] = np.asarray(bh, np.float32).T

    maskf = np.asarray(mask_ids).astype(np.float32)
    in_maps = []
    for c in range(8):
        b = c // 2
        mrep = np.ascontiguousarray(
            np.broadcast_to(maskf[b][None, :], (HID, E)).astype(BF16_NP))
        in_maps.append(dict(
            adjA=np.ascontiguousarray(adjA[b]),
            adjB=np.ascontiguousarray(adjB[b]), wpack=wpack,
            maskrep=mrep, fpack=fpack))
    return in_maps


def run(inputs, trace=False):
    nc = get_nc()
    in_maps = make_in_maps(**{k: np.asarray(v) for k, v in inputs.items()})
    res = bass_utils.run_bass_kernel_spmd(
        nc, in_maps, core_ids=list(range(8)), trace=trace
    )
    out = np.stack(
        [np.asarray(res.results[2 * b]["graph"]).reshape(HID) for b in range(B)]
    ).astype(np.float32)
    return out, res


def kernel(**inputs):
    out, _ = run(inputs, trace=False)
    return out
